# revision 36
# baseline (speedup 1.0000x reference)
"""Trainium2 Bass kernel for nn_CMLITargetLoss (CMLI target loss).

Data-parallel over batch: 64 samples -> 8 NeuronCores x 8 samples.

Host ships, per core (fp8_e4m3, D-major transposed local batch):
  textT/targetT/imageT : [768, 8, 197]  (text pre-masked: kept tokens only,
                          t=0 cls column always kept; masked token columns
                          are zero, so their sim rows vanish and the masked
                          sum of ||t||^2 is just the plain sum of squares)
  identH               : [128, 128] f16 identity (for the rinv row-broadcast)
Device computes per-core partial sums (one [128, 33] f32 tile); host combines.

Numerics (validated on host: rel_err ~7e-4 vs f32 reference, gate 2e-2):
  fp8 inputs; sim = t^T g per sample via fp8 DoubleRow matmuls (f32 psum);
  r2 = sum_d g^2 from exact f16 squares via ones-matmul row reduce;
  rinv = f16(1/sqrt(r2)); scaled sim = sim * rinv (TTR, f32, fused rowmax);
  A = sum_n (scaled>=m)*sim_raw (STT, exact one-hot since scaled kept f32);
  q = A/max(m,eps) = |g_sel|  =>  per-token loss term  ||t||^2 + q^2 - 2A.
  image loss via  sum i^2 - 2 sum i*g + sum g^2  (all accum_out reductions).
"""

import os
import sys

import numpy as np

for _p in ("/opt/trn_rl_repo", "/root/.axon_site/_ro/trn_rl_repo"):
    if os.path.isdir(_p) and _p not in sys.path:
        sys.path.insert(0, _p)

B, T, D = 64, 197, 768
NC_ = 8            # cores
BL = B // NC_      # 8 local samples per core
KD = D // 128      # 6 d-chunks
TM1 = T - 1        # 196
C0, C1 = 128, TM1 - 128   # token chunks: t = 1..128, 129..196
N1 = T - 128       # 69: patch-index chunk n = 128..196

_CACHE = {}

# engine assignment for the big elementwise passes (tuned so each engine's
# busy total is ~16-17us: ACT rate .833+accum, DVE 1.04, POOL .833+launch)
ENG_G2 = ("dve", "act", "pool", "dve", "act", "pool")  # 6 k-singles
ENG_EXT = "pool"                       # argmax A-extraction STTs

# partials column layout
PCOL_Q2 = 0
PCOL_A0 = 1    # ..8   per-sample A columns, token chunk 0
PCOL_A1 = 9    # ..16  token chunk 1
PCOL_T2 = 17   # ..22  (up to 6 partial-sum columns)
PCOL_G2 = 23   # ..28  (6 singles)
PCOL_I2 = 29   # ..36  (up to 8)
PCOL_IG = 37   # ..48  (up to 12 half-chunk columns)
PCOL_CLS = 49
NPCOL = 50


def _build():
    import concourse.bacc as bacc
    import concourse.tile as tile
    from concourse import mybir
    from contextlib import ExitStack

    f32 = mybir.dt.float32
    f16 = mybir.dt.float16
    fp8 = mybir.dt.float8e4
    Alu = mybir.AluOpType
    Act = mybir.ActivationFunctionType
    DR = mybir.MatmulPerfMode.DoubleRow

    nc = bacc.Bacc("TRN2", target_bir_lowering=False, debug=False)

    tT = nc.dram_tensor("textT", (D, BL, T), fp8, kind="ExternalInput")
    gT = nc.dram_tensor("targetT", (D, BL, T), fp8, kind="ExternalInput")
    iT = nc.dram_tensor("imageT", (D, BL, T), fp8, kind="ExternalInput")
    identH = nc.dram_tensor("identH", (128, 128), f16, kind="ExternalInput")
    outp = nc.dram_tensor("partials", (128, NPCOL), f32, kind="ExternalOutput")

    with tile.TileContext(nc) as tc, ExitStack() as ctx:
        consts = ctx.enter_context(tc.tile_pool(name="consts", bufs=1))
        inputs = ctx.enter_context(tc.tile_pool(name="inputs", bufs=1))
        sbuf = ctx.enter_context(tc.tile_pool(name="sbuf", bufs=1))
        scratch = ctx.enter_context(tc.tile_pool(name="scratch", bufs=2))
        psum = ctx.enter_context(tc.tile_pool(name="psum", bufs=1, space="PSUM"))

        eng = {"dve": nc.vector, "act": nc.scalar, "pool": nc.gpsimd}

        ident = consts.tile([128, 128], f16, tag="ident", name="ident")
        nc.sync.dma_start(out=ident, in_=identH[:, :])
        ones_col = consts.tile([128, 1], f16, tag="ones_col", name="ones_col")
        nc.vector.memset(ones_col, 1.0)
        ones_row = consts.tile([1, 128], f32, tag="ones_row", name="ones_row")
        nc.vector.memset(ones_row, 1.0)

        ost = consts.tile([128, NPCOL], f32, tag="ost", name="ost")
        nc.vector.memset(ost, 0.0)

        # ACT table preload (sqrt set also holds Square) + PE pstate ramp.
        actwarm = consts.tile([1, 1], f32, tag="actwarm", name="actwarm")
        nc.vector.memset(actwarm, 1.0)
        actwarm2 = consts.tile([1, 1], f32, tag="actwarm2", name="actwarm2")
        nc.scalar.activation(actwarm2, actwarm, Act.Sqrt)
        warm_ps = psum.tile([128, 2, 196], f32, tag="simc0", bufs=2, name="warm_ps")
        for w in range(30):
            nc.tensor.matmul(warm_ps[:, 0, 0:128], ident, ident,
                             start=True, stop=True, skip_group_check=True)

        # resident inputs [128, k, b, t]
        t_all = inputs.tile([128, KD, BL, T], fp8, tag="t_all", name="t_all")
        g_all = inputs.tile([128, KD, BL, T], fp8, tag="g_all", name="g_all")
        i_all = inputs.tile([128, KD, BL, T], fp8, tag="i_all", name="i_all")

        def _ld(dst, srcdram, k):
            nc.sync.dma_start(
                out=dst[:, k].rearrange("p b t -> p (b t)"),
                in_=srcdram[k * 128:(k + 1) * 128].rearrange("p b t -> p (b t)"))

        # g first (feeds the rinv critical chain), first two image chunks
        # next (feeds early i*g / i^2 work during the rinv wait), then text
        # (sim needs all of it), then the rest of image.
        for k in range(KD):
            _ld(g_all, gT, k)
        for k in (0, 1):
            _ld(i_all, iT, k)
        for k in range(KD):
            _ld(t_all, tT, k)
        for k in (2, 3, 4, 5):
            _ld(i_all, iT, k)

        # ---------------- target squares (f16, exact) + Sum g^2 -------------
        gsq = []
        for k in range(KD):
            gq = sbuf.tile([128, BL, T], f16, tag=f"gsq{k}", name=f"gsq{k}")
            gsq.append(gq)
            e = ENG_G2[k]
            if e == "act":
                nc.scalar.activation(gq, g_all[:, k], Act.Square,
                                     accum_out=ost[:, PCOL_G2 + k:PCOL_G2 + k + 1])
            else:
                eng[e].scalar_tensor_tensor(
                    out=gq, in0=g_all[:, k], scalar=1.0,
                    in1=g_all[:, k], op0=Alu.mult, op1=Alu.mult,
                    accum_out=ost[:, PCOL_G2 + k:PCOL_G2 + k + 1])

        # rows r2[b, n] = sum_d g^2 : ones-matmul reduce, 4 psum tiles of
        # 2 samples each, accumulated over the 6 square chunks.
        NROW = 2 * T
        r2row_ps = [psum.tile([1, NROW], f32, tag=f"rr{j}", name=f"r2row_ps{j}")
                    for j in range(4)]
        for k in range(KD):
            for j in range(4):
                nc.tensor.matmul(
                    r2row_ps[j], ones_col, gsq[k][:, 2 * j:2 * j + 2, :],
                    start=(k == 0), stop=(k == KD - 1))
        r2row = sbuf.tile([1, BL * T], f32, tag="r2row", name="r2row")
        with tc.high_priority():
            for j in range(4):
                nc.gpsimd.tensor_copy(r2row[:, j * NROW:(j + 1) * NROW],
                                      r2row_ps[j])

        # rows -> columns, rinv = f16(1/sqrt(r2))
        with tc.high_priority():
            r2c0_ps = psum.tile([128, BL], f32, tag="rr0", name="r2c0_ps")
            r2c1_ps = psum.tile([N1, BL], f32, tag="rr1", name="r2c1_ps")
            for b in range(BL):
                nc.tensor.matmul(r2c0_ps[:, b:b + 1], r2row[:, b * T:b * T + 128],
                                 ones_row[:, 0:1], skip_group_check=True)
                nc.tensor.matmul(r2c1_ps[:, b:b + 1],
                                 r2row[:, b * T + 128:(b + 1) * T],
                                 ones_row[:, 0:1], skip_group_check=True)
            r0a = sbuf.tile([128, BL], f32, tag="r0a", name="r0a")
            nc.scalar.activation(r0a, r2c0_ps, Act.Sqrt)
            r0b = sbuf.tile([N1, BL], f32, tag="r0b", name="r0b")
            nc.scalar.activation(r0b, r2c1_ps, Act.Sqrt)
            rinv0 = sbuf.tile([128, BL], f16, tag="rinv0", name="rinv0")
            rinv1 = sbuf.tile([N1, BL], f16, tag="rinv1", name="rinv1")
            with nc.allow_low_precision(reason="rinv f16: 1e-4 scale noise ok"):
                nc.vector.reciprocal(rinv0, r0a)
                nc.vector.reciprocal(rinv1, r0b)

        # ---------------- similarity + argmax extraction ---------------------
        Mcol = []
        for ci, P in ((0, 128), (1, C1)):
            mc = sbuf.tile([128, BL], f32, tag=f"mcol{ci}", name=f"mcol{ci}")
            nc.vector.memset(mc, 0.0)
            Mcol.append(mc)

        with tc.high_priority(offset=100000):
            for grp in range(BL // 2):
                b0 = 2 * grp
                pc0 = psum.tile([128, 2, TM1], f32, tag="simc0", bufs=2,
                                name=f"pc0_{grp}")
                pc1 = psum.tile([C1, 2, TM1], f32, tag="simc1", bufs=2,
                                name=f"pc1_{grp}")
                reps = []
                for g in range(2):
                    b = b0 + g
                    for j in range(3):
                        nc.tensor.matmul(
                            pc0[:, g, :], t_all[:, 2 * j:2 * j + 2, b, 1:1 + C0],
                            g_all[:, 2 * j:2 * j + 2, b, 1:T],
                            start=(j == 0), stop=(j == 2), perf_mode=DR)
                    for j in range(3):
                        nc.tensor.matmul(
                            pc1[:, g, :], t_all[:, 2 * j:2 * j + 2, b, 1 + C0:T],
                            g_all[:, 2 * j:2 * j + 2, b, 1:T],
                            start=(j == 0), stop=(j == 2), perf_mode=DR)
                    rep_ps = psum.tile([128, 200], f32, tag=f"rr{2 + (b % 2)}",
                                       name=f"rep_{b}")
                    nc.tensor.matmul(rep_ps[:, 0:128],
                                     rinv0[:, b:b + 1].broadcast_to([128, 128]),
                                     ident, skip_group_check=True)
                    nc.tensor.matmul(rep_ps[:, 128:T],
                                     rinv1[:, b:b + 1].broadcast_to([N1, 128]),
                                     ident[:N1, :N1], skip_group_check=True)
                    reps.append(rep_ps)
                for g in range(2):
                    b = b0 + g
                    rep_ps = reps[g]
                    for ci, (P, pc) in enumerate(((128, pc0), (C1, pc1))):
                        sc = scratch.tile([128, TM1], f32, tag="sc", bufs=4,
                                          name=f"sc{b}_{ci}")
                        nc.vector.tensor_tensor_reduce(
                            out=sc[:P], in0=pc[:P, g, :], in1=rep_ps[:P, 1:T],
                            scale=1.0, scalar=0.0, op0=Alu.mult, op1=Alu.max,
                            accum_out=Mcol[ci][:P, b:b + 1])
                        js = scratch.tile([128, TM1], f16, tag="js", bufs=4,
                                          name=f"js{b}_{ci}")
                        acol = PCOL_A0 if ci == 0 else PCOL_A1
                        eng[ENG_EXT].scalar_tensor_tensor(
                            out=js[:P], in0=sc[:P], scalar=Mcol[ci][:P, b:b + 1],
                            in1=pc[:P, g, :], op0=Alu.is_ge, op1=Alu.mult,
                            accum_out=ost[:P, acol + b:acol + b + 1])

        # ---------------- column math part 1 (DVE, before its late passes) ---
        qq = sbuf.tile([128, 2, BL], f32, tag="qq", name="qq")
        nc.vector.memset(qq, 0.0)
        for ci in range(2):
            mcl = scratch.tile([128, BL], f32, tag="colm", bufs=4, name=f"mcl{ci}")
            nc.vector.tensor_scalar(out=mcl, in0=Mcol[ci], scalar1=1e-20,
                                    scalar2=None, op0=Alu.max)
            rm = scratch.tile([128, BL], f32, tag="colm", bufs=4, name=f"rm{ci}")
            nc.vector.reciprocal(rm, mcl)
            acol = PCOL_A0 if ci == 0 else PCOL_A1
            nc.vector.tensor_mul(qq[:, ci], ost[:, acol:acol + BL], rm)

        # ---------------- late passes: text/image squares, image*target ------
        # ACT ops stay big (its queue drains in order); DVE/POOL ops are
        # half-chunk quanta so a greedy pick never delays the argmax wave
        # by more than ~0.7us.
        colctr = {}

        def _col(base):
            c = colctr.get(base, 0)
            colctr[base] = c + 1
            return ost[:, base + c:base + c + 1]

        def _sq(e, src, base, nm):
            halves = [src] if e == "act" else [src[:, 0:4], src[:, 4:8]]
            for hi, s in enumerate(halves):
                j = scratch.tile(list(s.shape), f16, tag="jsq", bufs=4,
                                 name=f"{nm}_{hi}")
                if e == "act":
                    nc.scalar.activation(j, s, Act.Square, accum_out=_col(base))
                else:
                    eng[e].scalar_tensor_tensor(
                        out=j, in0=s, scalar=1.0, in1=s,
                        op0=Alu.mult, op1=Alu.mult, accum_out=_col(base))

        def _mulacc(e, a, b, base, nm):
            for hi, (s0, s1) in enumerate(((a[:, 0:4], b[:, 0:4]),
                                           (a[:, 4:8], b[:, 4:8]))):
                j = scratch.tile(list(s0.shape), f16, tag="jsq", bufs=4,
                                 name=f"{nm}_{hi}")
                eng[e].scalar_tensor_tensor(
                    out=j, in0=s0, scalar=1.0, in1=s1,
                    op0=Alu.mult, op1=Alu.mult, accum_out=_col(base))

        # interleave by DMA arrival order so ready work exists at all times
        _mulacc("pool", i_all[:, 0], g_all[:, 0], PCOL_IG, "ig0")   # i0 early
        _sq("pool", i_all[:, 0], PCOL_I2, "i2a0")
        _sq("pool", i_all[:, 1], PCOL_I2, "i2a1")
        _mulacc("dve", i_all[:, 1], g_all[:, 1], PCOL_IG, "ig1")
        _sq("act", t_all[:, 0:2, :, 1:T], PCOL_T2, "t2a")
        _sq("act", t_all[:, 2:4, :, 1:T], PCOL_T2, "t2b")
        _sq("dve", t_all[:, 4, :, 1:T], PCOL_T2, "t2c")
        _sq("pool", t_all[:, 5, :, 1:T], PCOL_T2, "t2d")
        _mulacc("pool", i_all[:, 2], g_all[:, 2], PCOL_IG, "ig2")
        _sq("act", i_all[:, 2], PCOL_I2, "i2b")
        _sq("dve", i_all[:, 3], PCOL_I2, "i2c")
        _mulacc("dve", i_all[:, 3], g_all[:, 3], PCOL_IG, "ig3")
        _sq("act", i_all[:, 4:6], PCOL_I2, "i2d")
        _mulacc("pool", i_all[:, 4], g_all[:, 4], PCOL_IG, "ig4")
        _mulacc("pool", i_all[:, 5], g_all[:, 5], PCOL_IG, "ig5")

        # ---------------- cls token loss -------------------------------------
        dcls = sbuf.tile([128, KD, BL], f32, tag="dcls", name="dcls")
        for k in range(KD):
            nc.vector.tensor_sub(dcls[:, k], t_all[:, k, :, 0], g_all[:, k, :, 0])
        jcls = sbuf.tile([128, KD, BL], f16, tag="jcls", name="jcls")
        nc.scalar.activation(jcls, dcls, Act.Square,
                             accum_out=ost[:, PCOL_CLS:PCOL_CLS + 1])

        # Sum q^2 (ACT, last)
        jq = sbuf.tile([128, 2, BL], f16, tag="jq", name="jq")
        nc.scalar.activation(jq, qq, Act.Square,
                             accum_out=ost[:, PCOL_Q2:PCOL_Q2 + 1])

        nc.sync.dma_start(out=outp[:, :], in_=ost)

    nc.compile()
    return nc


def _get_nc():
    if "nc" not in _CACHE:
        _CACHE["nc"] = _build()
    return _CACHE["nc"]


def _prepare(image, text, target, padding_mask):
    import ml_dtypes

    fp8 = ml_dtypes.float8_e4m3
    image = np.asarray(image, dtype=np.float32)
    text = np.asarray(text, dtype=np.float32)
    target = np.asarray(target, dtype=np.float32)
    mask = np.asarray(padding_mask)

    keep = (mask[:, 1:] == 0)          # [B, 196] bool
    n_tokens = float(keep.sum())

    tq = text.astype(fp8)
    tq[:, 1:][~keep] = fp8(0.0)        # pre-mask dropped tokens (cls kept)
    gq = target.astype(fp8)
    iq = image.astype(fp8)
    identH = np.eye(128, dtype=np.float16)

    in_maps = []
    for c in range(NC_):
        sl = slice(c * BL, (c + 1) * BL)
        in_maps.append({
            "textT": np.ascontiguousarray(tq[sl].transpose(2, 0, 1)),
            "targetT": np.ascontiguousarray(gq[sl].transpose(2, 0, 1)),
            "imageT": np.ascontiguousarray(iq[sl].transpose(2, 0, 1)),
            "identH": identH,
        })
    return in_maps, n_tokens


def _combine(results, n_tokens):
    S_q2 = S_A = S_t2 = S_g2 = S_i2 = S_ig = S_cls = 0.0
    for r in results:
        P = r["partials"].astype(np.float64)
        S_q2 += P[:, PCOL_Q2].sum()
        S_A += P[:, PCOL_A0:PCOL_A0 + 2 * BL].sum()
        S_t2 += P[:, PCOL_T2:PCOL_T2 + 6].sum()
        S_g2 += P[:, PCOL_G2:PCOL_G2 + 6].sum()
        S_i2 += P[:, PCOL_I2:PCOL_I2 + 8].sum()
        S_ig += P[:, PCOL_IG:PCOL_IG + 12].sum()
        S_cls += P[:, PCOL_CLS].sum()

    S1 = S_t2 + S_q2 - 2.0 * S_A
    kd_tok = S1 / (n_tokens * D)
    kd_cls = S_cls / (B * D)
    kd_text = (n_tokens * kd_tok + kd_cls) / (n_tokens + 1.0)
    kd_img = (S_i2 - 2.0 * S_ig + S_g2) / (B * T * D)
    return np.float32((kd_text + kd_img) / 2.0)


def kernel(image, text, target, padding_mask):
    from concourse.bass_utils import run_bass_kernel_spmd

    in_maps, n_tokens = _prepare(image, text, target, padding_mask)
    nc = _get_nc()
    results = run_bass_kernel_spmd(nc, in_maps, core_ids=list(range(NC_))).results
    return _combine(results, n_tokens)


# revision 38
# speedup vs baseline: 1.0755x; 1.0755x over previous
"""Trainium2 Bass kernel for nn_CMLITargetLoss (CMLI target loss).

Data-parallel over batch: 64 samples -> 8 NeuronCores x 8 samples.

Host ships, per core (fp8_e4m3, D-major transposed local batch):
  textT/targetT/imageT : [768, 8, 197]  (text pre-masked: kept tokens only,
                          t=0 cls column always kept; masked token columns
                          are zero, so their sim rows vanish and the masked
                          sum of ||t||^2 is just the plain sum of squares)
  identH               : [128, 128] f16 identity (for the rinv row-broadcast)
Device computes per-core partial sums (one [128, 33] f32 tile); host combines.

Numerics (validated on host: rel_err ~7e-4 vs f32 reference, gate 2e-2):
  fp8 inputs; sim = t^T g per sample via fp8 DoubleRow matmuls (f32 psum);
  r2 = sum_d g^2 from exact f16 squares via ones-matmul row reduce;
  rinv = f16(1/sqrt(r2)); scaled sim = sim * rinv (TTR, f32, fused rowmax);
  A = sum_n (scaled>=m)*sim_raw (STT, exact one-hot since scaled kept f32);
  q = A/max(m,eps) = |g_sel|  =>  per-token loss term  ||t||^2 + q^2 - 2A.
  image loss via  sum i^2 - 2 sum i*g + sum g^2  (all accum_out reductions).
"""

import os
import sys

import numpy as np

for _p in ("/opt/trn_rl_repo", "/root/.axon_site/_ro/trn_rl_repo"):
    if os.path.isdir(_p) and _p not in sys.path:
        sys.path.insert(0, _p)

B, T, D = 64, 197, 768
NC_ = 8            # cores
BL = B // NC_      # 8 local samples per core
KD = D // 128      # 6 d-chunks
TM1 = T - 1        # 196
C0, C1 = 128, TM1 - 128   # token chunks: t = 1..128, 129..196
N1 = T - 128       # 69: patch-index chunk n = 128..196

_CACHE = {}

# engine assignment for the big elementwise passes (tuned so each engine's
# busy total is ~16-17us: ACT rate .833+accum, DVE 1.04, POOL .833+launch)
ENG_G2 = ("dve", "act", "pool", "dve", "act", "pool")  # 6 k-singles
ENG_EXT = "pool"                       # argmax A-extraction STTs

# partials column layout
PCOL_Q2 = 0
PCOL_A0 = 1    # ..8   per-sample A columns, token chunk 0
PCOL_A1 = 9    # ..16  token chunk 1
PCOL_T2 = 17   # ..22  (up to 6 partial-sum columns)
PCOL_G2 = 23   # ..28  (6 singles)
PCOL_I2 = 29   # ..36  (up to 8)
PCOL_IG = 37   # ..48  (up to 12 half-chunk columns)
PCOL_CLS = 49
NPCOL = 50


def _build():
    import concourse.bacc as bacc
    import concourse.tile as tile
    from concourse import mybir
    from contextlib import ExitStack

    f32 = mybir.dt.float32
    f16 = mybir.dt.float16
    fp8 = mybir.dt.float8e4
    Alu = mybir.AluOpType
    Act = mybir.ActivationFunctionType
    DR = mybir.MatmulPerfMode.DoubleRow

    nc = bacc.Bacc("TRN2", target_bir_lowering=False, debug=False)

    tT = nc.dram_tensor("textT", (D, BL, T), fp8, kind="ExternalInput")
    gT = nc.dram_tensor("targetT", (D, BL, T), fp8, kind="ExternalInput")
    iT = nc.dram_tensor("imageT", (D, BL, T), fp8, kind="ExternalInput")
    identH = nc.dram_tensor("identH", (128, 128), f16, kind="ExternalInput")
    outp = nc.dram_tensor("partials", (128, NPCOL), f32, kind="ExternalOutput")

    with tile.TileContext(nc) as tc, ExitStack() as ctx:
        consts = ctx.enter_context(tc.tile_pool(name="consts", bufs=1))
        inputs = ctx.enter_context(tc.tile_pool(name="inputs", bufs=1))
        sbuf = ctx.enter_context(tc.tile_pool(name="sbuf", bufs=1))
        scratch = ctx.enter_context(tc.tile_pool(name="scratch", bufs=2))
        psum = ctx.enter_context(tc.tile_pool(name="psum", bufs=1, space="PSUM"))

        eng = {"dve": nc.vector, "act": nc.scalar, "pool": nc.gpsimd}

        ident = consts.tile([128, 128], f16, tag="ident", name="ident")
        nc.sync.dma_start(out=ident, in_=identH[:, :])
        ones_col = consts.tile([128, 1], f16, tag="ones_col", name="ones_col")
        nc.vector.memset(ones_col, 1.0)
        ones_row = consts.tile([1, 128], f32, tag="ones_row", name="ones_row")
        nc.vector.memset(ones_row, 1.0)

        ost = consts.tile([128, NPCOL], f32, tag="ost", name="ost")
        nc.vector.memset(ost, 0.0)

        # ACT table preload (sqrt set also holds Square) + PE pstate ramp.
        actwarm = consts.tile([1, 1], f32, tag="actwarm", name="actwarm")
        nc.vector.memset(actwarm, 1.0)
        actwarm2 = consts.tile([1, 1], f32, tag="actwarm2", name="actwarm2")
        nc.scalar.activation(actwarm2, actwarm, Act.Sqrt)
        warm_ps = psum.tile([128, 2, 196], f32, tag="simc0", bufs=2, name="warm_ps")
        for w in range(30):
            nc.tensor.matmul(warm_ps[:, 0, 0:128], ident, ident,
                             start=True, stop=True, skip_group_check=True)

        # resident inputs [128, k, b, t]
        t_all = inputs.tile([128, KD, BL, T], fp8, tag="t_all", name="t_all")
        g_all = inputs.tile([128, KD, BL, T], fp8, tag="g_all", name="g_all")
        i_all = inputs.tile([128, KD, BL, T], fp8, tag="i_all", name="i_all")

        def _ld(dst, srcdram, k):
            nc.sync.dma_start(
                out=dst[:, k].rearrange("p b t -> p (b t)"),
                in_=srcdram[k * 128:(k + 1) * 128].rearrange("p b t -> p (b t)"))

        # g first (feeds the rinv critical chain), first two image chunks
        # next (feeds early i*g / i^2 work during the rinv wait), then text
        # (sim needs all of it), then the rest of image.
        for k in range(KD):
            _ld(g_all, gT, k)
        for k in (0, 1):
            _ld(i_all, iT, k)
        for k in range(KD):
            _ld(t_all, tT, k)
        for k in (2, 3, 4, 5):
            _ld(i_all, iT, k)

        # ---------------- target squares (f16, exact) + Sum g^2 -------------
        gsq = []
        for k in range(KD):
            gq = sbuf.tile([128, BL, T], f16, tag=f"gsq{k}", name=f"gsq{k}")
            gsq.append(gq)
            e = ENG_G2[k]
            if e == "act":
                nc.scalar.activation(gq, g_all[:, k], Act.Square,
                                     accum_out=ost[:, PCOL_G2 + k:PCOL_G2 + k + 1])
            else:
                eng[e].scalar_tensor_tensor(
                    out=gq, in0=g_all[:, k], scalar=1.0,
                    in1=g_all[:, k], op0=Alu.mult, op1=Alu.mult,
                    accum_out=ost[:, PCOL_G2 + k:PCOL_G2 + k + 1])

        # rows r2[b, n] = sum_d g^2 : ones-matmul reduce, 4 psum tiles of
        # 2 samples each, accumulated over the 6 square chunks.
        NROW = 2 * T
        r2row_ps = [psum.tile([1, NROW], f32, tag=f"rr{j}", name=f"r2row_ps{j}")
                    for j in range(4)]
        for k in range(KD):
            for j in range(4):
                nc.tensor.matmul(
                    r2row_ps[j], ones_col, gsq[k][:, 2 * j:2 * j + 2, :],
                    start=(k == 0), stop=(k == KD - 1))
        r2row = sbuf.tile([1, BL * T], f32, tag="r2row", name="r2row")
        with tc.high_priority():
            for j in range(4):
                e = nc.gpsimd if j % 2 == 0 else nc.vector
                e.tensor_copy(r2row[:, j * NROW:(j + 1) * NROW], r2row_ps[j])

        # rows -> columns, rinv = f16(1/sqrt(r2))
        with tc.high_priority():
            r2c0_ps = psum.tile([128, BL], f32, tag="rr0", name="r2c0_ps")
            r2c1_ps = psum.tile([N1, BL], f32, tag="rr1", name="r2c1_ps")
            for b in range(BL):
                nc.tensor.matmul(r2c0_ps[:, b:b + 1], r2row[:, b * T:b * T + 128],
                                 ones_row[:, 0:1], skip_group_check=True)
                nc.tensor.matmul(r2c1_ps[:, b:b + 1],
                                 r2row[:, b * T + 128:(b + 1) * T],
                                 ones_row[:, 0:1], skip_group_check=True)
            r0a = sbuf.tile([128, BL], f32, tag="r0a", name="r0a")
            nc.scalar.activation(r0a, r2c0_ps, Act.Sqrt)
            r0b = sbuf.tile([N1, BL], f32, tag="r0b", name="r0b")
            nc.scalar.activation(r0b, r2c1_ps, Act.Sqrt)
            rinv0 = sbuf.tile([128, BL], f16, tag="rinv0", name="rinv0")
            rinv1 = sbuf.tile([N1, BL], f16, tag="rinv1", name="rinv1")
            with nc.allow_low_precision(reason="rinv f16: 1e-4 scale noise ok"):
                nc.vector.reciprocal(rinv0, r0a)
                nc.vector.reciprocal(rinv1, r0b)

        # ---------------- similarity + argmax extraction ---------------------
        Mcol = []
        for ci, P in ((0, 128), (1, C1)):
            mc = sbuf.tile([128, BL], f32, tag=f"mcol{ci}", name=f"mcol{ci}")
            nc.vector.memset(mc, 0.0)
            Mcol.append(mc)

        with tc.high_priority(offset=100000):
            for grp in range(BL // 2):
                b0 = 2 * grp
                pc0 = psum.tile([128, 2, TM1], f32, tag="simc0", bufs=2,
                                name=f"pc0_{grp}")
                pc1 = psum.tile([C1, 2, TM1], f32, tag="simc1", bufs=2,
                                name=f"pc1_{grp}")
                reps = []
                for g in range(2):
                    b = b0 + g
                    for j in range(3):
                        nc.tensor.matmul(
                            pc0[:, g, :], t_all[:, 2 * j:2 * j + 2, b, 1:1 + C0],
                            g_all[:, 2 * j:2 * j + 2, b, 1:T],
                            start=(j == 0), stop=(j == 2), perf_mode=DR)
                    for j in range(3):
                        nc.tensor.matmul(
                            pc1[:, g, :], t_all[:, 2 * j:2 * j + 2, b, 1 + C0:T],
                            g_all[:, 2 * j:2 * j + 2, b, 1:T],
                            start=(j == 0), stop=(j == 2), perf_mode=DR)
                    rep_ps = psum.tile([128, 200], f32, tag=f"rr{2 + (b % 2)}",
                                       name=f"rep_{b}")
                    nc.tensor.matmul(rep_ps[:, 0:128],
                                     rinv0[:, b:b + 1].broadcast_to([128, 128]),
                                     ident, skip_group_check=True)
                    nc.tensor.matmul(rep_ps[:, 128:T],
                                     rinv1[:, b:b + 1].broadcast_to([N1, 128]),
                                     ident[:N1, :N1], skip_group_check=True)
                    reps.append(rep_ps)
                for g in range(2):
                    b = b0 + g
                    rep_ps = reps[g]
                    for ci, (P, pc) in enumerate(((128, pc0), (C1, pc1))):
                        sc = scratch.tile([128, TM1], f32, tag="sc", bufs=4,
                                          name=f"sc{b}_{ci}")
                        nc.vector.tensor_tensor_reduce(
                            out=sc[:P], in0=pc[:P, g, :], in1=rep_ps[:P, 1:T],
                            scale=1.0, scalar=0.0, op0=Alu.mult, op1=Alu.max,
                            accum_out=Mcol[ci][:P, b:b + 1])
                        js = scratch.tile([128, TM1], f16, tag="js", bufs=4,
                                          name=f"js{b}_{ci}")
                        acol = PCOL_A0 if ci == 0 else PCOL_A1
                        eng[ENG_EXT].scalar_tensor_tensor(
                            out=js[:P], in0=sc[:P], scalar=Mcol[ci][:P, b:b + 1],
                            in1=pc[:P, g, :], op0=Alu.is_ge, op1=Alu.mult,
                            accum_out=ost[:P, acol + b:acol + b + 1])

        # ---------------- column math part 1 (DVE, before its late passes) ---
        qq = sbuf.tile([128, 2, BL], f32, tag="qq", name="qq")
        nc.vector.memset(qq, 0.0)
        for ci in range(2):
            mcl = scratch.tile([128, BL], f32, tag="colm", bufs=4, name=f"mcl{ci}")
            nc.vector.tensor_scalar(out=mcl, in0=Mcol[ci], scalar1=1e-20,
                                    scalar2=None, op0=Alu.max)
            rm = scratch.tile([128, BL], f32, tag="colm", bufs=4, name=f"rm{ci}")
            nc.vector.reciprocal(rm, mcl)
            acol = PCOL_A0 if ci == 0 else PCOL_A1
            nc.vector.tensor_mul(qq[:, ci], ost[:, acol:acol + BL], rm)

        # ---------------- late passes: text/image squares, image*target ------
        # ACT ops stay big (its queue drains in order); DVE/POOL ops are
        # half-chunk quanta so a greedy pick never delays the argmax wave
        # by more than ~0.7us.
        colctr = {}

        def _col(base):
            c = colctr.get(base, 0)
            colctr[base] = c + 1
            return ost[:, base + c:base + c + 1]

        def _sq(e, src, base, nm):
            halves = [src] if e == "act" else [src[:, 0:4], src[:, 4:8]]
            for hi, s in enumerate(halves):
                j = scratch.tile(list(s.shape), f16, tag="jsq", bufs=4,
                                 name=f"{nm}_{hi}")
                if e == "act":
                    nc.scalar.activation(j, s, Act.Square, accum_out=_col(base))
                else:
                    eng[e].scalar_tensor_tensor(
                        out=j, in0=s, scalar=1.0, in1=s,
                        op0=Alu.mult, op1=Alu.mult, accum_out=_col(base))

        def _mulacc(e, a, b, base, nm):
            for hi, (s0, s1) in enumerate(((a[:, 0:4], b[:, 0:4]),
                                           (a[:, 4:8], b[:, 4:8]))):
                j = scratch.tile(list(s0.shape), f16, tag="jsq", bufs=4,
                                 name=f"{nm}_{hi}")
                eng[e].scalar_tensor_tensor(
                    out=j, in0=s0, scalar=1.0, in1=s1,
                    op0=Alu.mult, op1=Alu.mult, accum_out=_col(base))

        # interleave by DMA arrival order so ready work exists at all times
        _mulacc("pool", i_all[:, 0], g_all[:, 0], PCOL_IG, "ig0")   # i0 early
        _sq("act", i_all[:, 0:2], PCOL_I2, "i2a")                    # ACT pair
        _mulacc("dve", i_all[:, 1], g_all[:, 1], PCOL_IG, "ig1")
        _sq("act", t_all[:, 0:2, :, 1:T], PCOL_T2, "t2a")
        _sq("act", t_all[:, 2:4, :, 1:T], PCOL_T2, "t2b")
        _sq("dve", t_all[:, 4, :, 1:T], PCOL_T2, "t2c")
        _sq("act", t_all[:, 5, :, 1:T], PCOL_T2, "t2d")
        _mulacc("pool", i_all[:, 2], g_all[:, 2], PCOL_IG, "ig2")
        _sq("act", i_all[:, 2], PCOL_I2, "i2b")
        _sq("dve", i_all[:, 3], PCOL_I2, "i2c")
        _mulacc("dve", i_all[:, 3], g_all[:, 3], PCOL_IG, "ig3")
        _sq("act", i_all[:, 4:6], PCOL_I2, "i2d")
        _mulacc("pool", i_all[:, 4], g_all[:, 4], PCOL_IG, "ig4")
        _mulacc("pool", i_all[:, 5], g_all[:, 5], PCOL_IG, "ig5")

        # ---------------- cls token loss -------------------------------------
        dcls = sbuf.tile([128, KD, BL], f32, tag="dcls", name="dcls")
        for k in range(KD):
            nc.vector.tensor_sub(dcls[:, k], t_all[:, k, :, 0], g_all[:, k, :, 0])
        jcls = sbuf.tile([128, KD, BL], f16, tag="jcls", name="jcls")
        nc.scalar.activation(jcls, dcls, Act.Square,
                             accum_out=ost[:, PCOL_CLS:PCOL_CLS + 1])

        # Sum q^2 (ACT, last)
        jq = sbuf.tile([128, 2, BL], f16, tag="jq", name="jq")
        nc.scalar.activation(jq, qq, Act.Square,
                             accum_out=ost[:, PCOL_Q2:PCOL_Q2 + 1])

        nc.sync.dma_start(out=outp[:, :], in_=ost)

    nc.compile()
    return nc


def _get_nc():
    if "nc" not in _CACHE:
        _CACHE["nc"] = _build()
    return _CACHE["nc"]


def _prepare(image, text, target, padding_mask):
    import ml_dtypes

    fp8 = ml_dtypes.float8_e4m3
    image = np.asarray(image, dtype=np.float32)
    text = np.asarray(text, dtype=np.float32)
    target = np.asarray(target, dtype=np.float32)
    mask = np.asarray(padding_mask)

    keep = (mask[:, 1:] == 0)          # [B, 196] bool
    n_tokens = float(keep.sum())

    tq = text.astype(fp8)
    tq[:, 1:][~keep] = fp8(0.0)        # pre-mask dropped tokens (cls kept)
    gq = target.astype(fp8)
    iq = image.astype(fp8)
    identH = np.eye(128, dtype=np.float16)

    in_maps = []
    for c in range(NC_):
        sl = slice(c * BL, (c + 1) * BL)
        in_maps.append({
            "textT": np.ascontiguousarray(tq[sl].transpose(2, 0, 1)),
            "targetT": np.ascontiguousarray(gq[sl].transpose(2, 0, 1)),
            "imageT": np.ascontiguousarray(iq[sl].transpose(2, 0, 1)),
            "identH": identH,
        })
    return in_maps, n_tokens


def _combine(results, n_tokens):
    S_q2 = S_A = S_t2 = S_g2 = S_i2 = S_ig = S_cls = 0.0
    for r in results:
        P = r["partials"].astype(np.float64)
        S_q2 += P[:, PCOL_Q2].sum()
        S_A += P[:, PCOL_A0:PCOL_A0 + 2 * BL].sum()
        S_t2 += P[:, PCOL_T2:PCOL_T2 + 6].sum()
        S_g2 += P[:, PCOL_G2:PCOL_G2 + 6].sum()
        S_i2 += P[:, PCOL_I2:PCOL_I2 + 8].sum()
        S_ig += P[:, PCOL_IG:PCOL_IG + 12].sum()
        S_cls += P[:, PCOL_CLS].sum()

    S1 = S_t2 + S_q2 - 2.0 * S_A
    kd_tok = S1 / (n_tokens * D)
    kd_cls = S_cls / (B * D)
    kd_text = (n_tokens * kd_tok + kd_cls) / (n_tokens + 1.0)
    kd_img = (S_i2 - 2.0 * S_ig + S_g2) / (B * T * D)
    return np.float32((kd_text + kd_img) / 2.0)


def kernel(image, text, target, padding_mask):
    from concourse.bass_utils import run_bass_kernel_spmd

    in_maps, n_tokens = _prepare(image, text, target, padding_mask)
    nc = _get_nc()
    results = run_bass_kernel_spmd(nc, in_maps, core_ids=list(range(NC_))).results
    return _combine(results, n_tokens)
